# revision 1
# baseline (speedup 1.0000x reference)
"""Fused transformer decoder layer (self-attn + cross-attn + FFN, 3 LayerNorms)
for Trainium2, SPMD across 8 NeuronCores.

Sharding: 2 cores per batch element (B=4). Each core owns 512 query rows of
its batch element, picked as four 128-row blocks interleaved so the causal
self-attention work is balanced across the pair ({0,3,4,7} / {1,2,5,6}).
K/V projections are computed redundantly per core (no collectives needed).

On-device layout: activations are kept feature-major ("transposed", [D, rows])
so every linear layer uses the weight matrices exactly as stored:
    out^T [Dout, r] = matmul(lhsT=W[D, Dout]-tile, rhs=act^T[D, r]-tile).
Attention scores are computed transposed (scores^T[k, q] = K·Q^T per head);
softmax denominators are harvested by augmenting V with 64 all-ones columns,
which lands the per-query sums partition-replicated next to the attnV output.
The causal mask is applied multiplicatively post-exp on the GpSimd engine.

Everything on the matmul path is bf16 (weights, activations, probs); LN
stats and softmax normalization run in fp32. Weights are host-repacked into
the on-device tile layout ([m-block, partition, k-block, col] order) inside
one flat DRAM tensor, so every weight DMA moves 1 MB with >=2 KB contiguous
per partition line. Small fp32 params ride in a second flat tensor.
Output is bf16.

build_nc(reps) emits the layer `reps` times in one program (one NEFF) so
dispatch overhead can be amortized over genuine back-to-back executions;
the grading entry point uses reps=1.
"""

import numpy as np
import ml_dtypes

import concourse.bacc as bacc
import concourse.bass as bass
import concourse.mybir as mybir
import concourse.tile as tile
from concourse.bass_utils import run_bass_kernel_spmd

F32 = mybir.dt.float32
F32R = mybir.dt.float32r
BF16 = mybir.dt.bfloat16
AF = mybir.ActivationFunctionType
ALU = mybir.AluOpType

B, S, D, DFF, H = 4, 1024, 1024, 4096, 16
R = 512                   # query rows per core
NK = D // 128             # 8 k-tiles over D
NP = H // 2               # 8 head pairs
NM2 = DFF // 128          # 32 m-tiles over DFF
EPS = 1e-3
NEG = -1e9
BLOCKS = [[0, 3, 4, 7], [1, 2, 5, 6]]   # 128-row q-blocks per half-core

# element offsets into the flat bf16 weight pack
_WOFF = {}
_off = 0
for _nm, _sz in (("wq1", D * D), ("wk1", D * D), ("wv1", D * D), ("wo1", D * D),
                 ("wq2", D * D), ("wk2", D * D), ("wv2", D * D), ("wo2", D * D),
                 ("w_ff1", D * DFF), ("w_ff2", DFF * D)):
    _WOFF[_nm] = _off
    _off += _sz
WPACK_N = _off

# element offsets into the flat f32 param pack
_VOFF = {}
_off = 0
for _nm, _sz in (("bq1", D), ("bk1", D), ("bo1", D),
                 ("bq2", D), ("bk2", D), ("bo2", D),
                 ("bv1", D), ("bv2", D),
                 ("b_ff1", DFF), ("b_ff2", D),
                 ("g1", D), ("be1", D), ("g2", D), ("be2", D),
                 ("g3", D), ("be3", D)):
    _VOFF[_nm] = _off
    _off += _sz
VPACK_N = _off

_NC_CACHE = {}


def _ln(nc, pools, y, gcol, bcol, out_tiles, ones_bf):
    """LayerNorm over the partition (feature) axis of 8 [128, R] bf16 tiles."""
    tmp = pools["lntmp"]
    # sum and sq-sum share the 2-bank psatt slot, keeping ps_gen free for
    # the next block's projections to overlap the LN tail.
    psln = pools["psatt"].tile([128, 2, R], F32, tag="ps_att", name="ln_ps")
    pssum, pssq = psln[:, 0, :], psln[:, 1, :]
    for m in range(NK):
        nc.tensor.matmul(pssum, ones_bf, y[m], start=(m == 0), stop=(m == NK - 1))
    for m in range(NK):
        sq = pools["sq"].tile([128, R], BF16, tag="ln_sqt", name="ln_sqt")
        nc.scalar.activation(sq, y[m], AF.Square)
        nc.tensor.matmul(pssq, ones_bf, sq, start=(m == 0), stop=(m == NK - 1))
    mean = tmp.tile([128, R], F32, tag="ln_mean", name="ln_mean")
    nc.vector.tensor_scalar_mul(mean, pssum, 1.0 / D)
    rv = tmp.tile([128, R], F32, tag="ln_sc", name="ln_rv")
    nc.vector.tensor_scalar_mul(rv, pssq, 1.0 / D)      # E[x^2]
    msq = tmp.tile([128, R], F32, tag="ln_t0", name="ln_msq")
    nc.vector.tensor_mul(msq, mean, mean)
    nc.vector.tensor_sub(rv, rv, msq)                   # var
    nc.scalar.activation(rv, rv, AF.Sqrt, bias=pools["epsc"][:, 0:1])
    nc.vector.reciprocal(rv, rv)                        # rstd (broadcast)
    nc.vector.tensor_mul(mean, mean, rv)                # mean*rstd (broadcast)
    for m in range(NK):
        t = tmp.tile([128, R], F32, tag=f"ln_t{m % 3}", name="ln_t")
        nc.vector.tensor_mul(t, y[m], rv)
        nc.vector.tensor_sub(t, t, mean)
        nc.vector.tensor_scalar(out_tiles[m], t, gcol[:, m:m + 1], bcol[:, m:m + 1],
                                ALU.mult, ALU.add)


def _attention(nc, pools, q_src, kv_src, resid, wq_h, wk_h, wv_h, wo_h,
               bqc, bkc, bv_ap, boc, mask_sb, y_out, ones_row, label):
    """One multi-head attention block + residual; writes pre-LN y_out tiles.

    wq_h/wk_h/wv_h/wo_h: per-projection "half" APs — lists of 2 DRAM views
    [128(p), 4(m), NK(k), 128(c)] covering m-blocks {4i..4i+3}.
    bv_ap: [1, D] f32r row of the V bias.
    """
    wpool, ps, tmp = pools["w"], pools["psum"], pools["atmp"]
    causal = mask_sb is not None

    # persistent V_aug for both pairs of the active quarter; ones columns
    # written once per attention.  layout [128, pair(2), s(NK), 256] with
    # per-s columns [v_h0(64) | ones(128) | v_h1(64)]
    vaug = pools["vaug"].tile([128, 2, NK, 256], BF16, tag="vaug",
                              name=f"vaug_{label}")
    nc.vector.memset(vaug[:, :, :, 64:192], 1.0)

    nh = []
    wk_cur = wq_cur = wv_cur = None
    for p in range(NP):
        if p % 4 == 0:
            i = p // 4
            wv_cur = wpool.tile([128, 4, NK, 128], BF16, tag="wq",
                                name=f"wvb_{label}_{i}")
            nc.sync.dma_start(out=wv_cur, in_=wv_h[i])
            wk_cur = wpool.tile([128, 4, NK, 128], BF16, tag="wq",
                                name=f"wkc_{label}_{i}")
            nc.sync.dma_start(out=wk_cur, in_=wk_h[i])
            wq_cur = wpool.tile([128, 4, NK, 128], BF16, tag="wq",
                                name=f"wqc_{label}_{i}")
            nc.sync.dma_start(out=wq_cur, in_=wq_h[i])
        if p % 2 == 0:
            # ---- V for two pairs produced together (moving dim 256) ----
            j = p // 2
            jl = j % 2
            bv_chunk = tmp.tile([1, 256], F32R, tag="bv_chunk",
                                name=f"bvc_{label}_{j}")
            nc.sync.dma_start(out=bv_chunk, in_=bv_ap[:, j * 256:(j + 1) * 256])
            for s in range(NK):
                psv = ps.tile([128, 256], F32, tag="ps_gen", name="ps_v")
                for k in range(NK):
                    nc.tensor.matmul(psv, kv_src[k][:, s * 128:(s + 1) * 128],
                                     wv_cur[:, 2 * jl:2 * jl + 2, k, :],
                                     start=(k == 0), stop=False)
                # bias row via K=1 matmul: psv[m, c] += 1 * bv[c]
                nc.tensor.matmul(psv, ones_row, bv_chunk[0:1, :],
                                 start=False, stop=True)
                # single copy drops both pairs' V into place around the ones
                dst = vaug[:, :, s, :].rearrange(
                    "p i (a c) -> p i a c", c=64)[:, :, 0:4:3, :]
                src = psv.rearrange("p (i a c) -> p i a c", a=2, c=64)
                nc.vector.tensor_copy(dst, src)
        pl = p % 4

        # ---- K^T and Q^T for this pair ----
        kt_p = pools["kt"].tile([128, S], BF16, tag="kt", name=f"kt_{label}_{p}")
        for half in range(2):
            psk = ps.tile([128, R], F32, tag="ps_gen", name="ps_k")
            for k in range(NK):
                nc.tensor.matmul(psk, wk_cur[:, pl, k, :],
                                 kv_src[k][:, half * 512:(half + 1) * 512],
                                 start=(k == 0), stop=(k == NK - 1))
            nc.scalar.activation(kt_p[:, half * 512:(half + 1) * 512], psk,
                                 AF.Identity, bias=bkc[:, p:p + 1])
        psq = ps.tile([128, R], F32, tag="ps_gen", name="ps_q")
        for k in range(NK):
            nc.tensor.matmul(psq, wq_cur[:, pl, k, :], q_src[k],
                             start=(k == 0), stop=(k == NK - 1))
        qt_p = pools["qt"].tile([128, R], BF16, tag="qt", name=f"qt_{label}_{p}")
        nc.scalar.activation(qt_p, psq, AF.Identity, bias=bqc[:, p:p + 1])

        # ---- scores^T, exp, attnV (softmax sums ride along in V_aug ones) --
        # Both heads share one 2-bank PSUM tile; exp runs once over both.
        va = vaug[:, p % 2]
        sscp = pools["pssc"].tile([128, 2, R], F32, tag="ps_sc", name="ps_sc")
        psap = pools["psatt"].tile([128, 2, R], F32, tag="ps_att", name="ps_att")
        for s in range(NK):
            c0 = 128 * (s // 2) if causal else 0
            n = R - c0
            nc.tensor.matmul(sscp[:, 0, c0:R], kt_p[0:64, s * 128:(s + 1) * 128],
                             qt_p[0:64, c0:R], start=True, stop=True)
            nc.tensor.matmul(sscp[:, 1, c0:R], kt_p[64:128, s * 128:(s + 1) * 128],
                             qt_p[64:128, c0:R], start=True, stop=True)
            e = pools["exp"].tile([128, 2, R], BF16, tag="e", name="e")
            nc.scalar.activation(e[:, :, c0:R], sscp[:, :, c0:R], AF.Exp,
                                 scale=0.125)
            if causal:
                # zero the future half of the diagonal q-block (binary mask,
                # duplicated per head) on the otherwise-idle GpSimd engine
                nc.gpsimd.tensor_mul(
                    e[:, :, c0:c0 + 128], e[:, :, c0:c0 + 128],
                    mask_sb[:, s, :].rearrange("p (h c) -> p h c", h=2))
            nc.tensor.matmul(psap[:, 0, c0:R], va[:, s, 0:128], e[:, 0, c0:R],
                             start=(s == 0), stop=(s == NK - 1),
                             skip_group_check=True)
            nc.tensor.matmul(psap[:, 1, c0:R], va[:, s, 128:256], e[:, 1, c0:R],
                             start=(s == 0), stop=(s == NK - 1),
                             skip_group_check=True)

        # evacuate the attnV accumulator promptly so the next pair's attnV can
        # reuse the psatt banks without waiting for the normalize chain
        pcp = pools["sq"].tile([128, 2, R], F32, tag="pcp", name="pcp")
        nc.vector.tensor_copy(pcp, psap)

        # normalize: head0 out rows 0:64 / sums 64:128; head1 sums 0:64 / out 64:128
        nh_p = pools["nh"].tile([128, R], BF16, tag=f"nh_{p}", name=f"nh_{label}_{p}")
        inv = tmp.tile([128, R], F32, tag="inv", name="inv")
        invs = tmp.tile([128, R], F32, tag="invs", name="invs")
        nc.vector.reciprocal(inv[64:128, :], pcp[64:128, 0, :])
        nc.sync.dma_start(out=invs[0:64, :], in_=inv[64:128, :])
        nc.vector.tensor_mul(nh_p[0:64, :], pcp[0:64, 0, :], invs[0:64, :])
        nc.vector.reciprocal(inv[0:64, :], pcp[0:64, 1, :])
        nc.sync.dma_start(out=invs[64:128, :], in_=inv[0:64, :])
        nc.vector.tensor_mul(nh_p[64:128, :], pcp[64:128, 1, :], invs[64:128, :])
        nh.append(nh_p)

    # ---- output projection + bias + residual ----
    wo_cur = None
    for m in range(NK):
        if m % 4 == 0:
            wo_cur = wpool.tile([128, 4, NK, 128], BF16, tag="wq",
                                name=f"woc_{label}_{m // 4}")
            nc.sync.dma_start(out=wo_cur, in_=wo_h[m // 4])
        pso = ps.tile([128, R], F32, tag="ps_gen", name="ps_o")
        for p in range(NP):
            nc.tensor.matmul(pso, wo_cur[:, m % 4, p, :], nh[p],
                             start=(p == 0), stop=(p == NP - 1))
        nc.vector.scalar_tensor_tensor(y_out[m], pso, boc[:, m:m + 1],
                                       resid[m], ALU.add, ALU.add)


def build_nc(reps=1):
    nc = bacc.Bacc("TRN2", target_bir_lowering=False, debug=False)

    xt = nc.dram_tensor("xt", [D, S], BF16, kind="ExternalInput")
    xq = nc.dram_tensor("xq", [D, R], BF16, kind="ExternalInput")
    enc = nc.dram_tensor("enc", [D, S], BF16, kind="ExternalInput")
    maskst = nc.dram_tensor("maskst", [128, NK, 256], BF16, kind="ExternalInput")
    wpack = nc.dram_tensor("wpack", [WPACK_N], BF16, kind="ExternalInput")
    vpack = nc.dram_tensor("vpack", [VPACK_N], F32, kind="ExternalInput")
    tick = nc.dram_tensor("tick", [1, 1], F32, kind="ExternalInput")
    out_t = nc.dram_tensor("out_t", [D, R], BF16, kind="ExternalOutput")

    def whalves(nm, nm_blocks):
        # host layout [m, p, k, c]; -> list of [128(p), 4(m), NK(k), 128(c)]
        full = wpack[_WOFF[nm]:_WOFF[nm] + nm_blocks * D * 128].rearrange(
            "(m p k c) -> p m k c", p=128, k=NK, c=128)
        return [full[:, 4 * i:4 * i + 4] for i in range(nm_blocks // 4)]

    def ff2_views():
        # host layout [m, q, p, k, c]; -> per m: [128, 4(q), NK, 128]
        full = wpack[_WOFF["w_ff2"]:_WOFF["w_ff2"] + DFF * D].rearrange(
            "(m q p k c) -> p m q k c", q=DFF // D, p=128, k=NK, c=128)
        return [full[:, m] for m in range(NK)]

    def vcol(nm, n=D):
        return vpack[_VOFF[nm]:_VOFF[nm] + n].rearrange("(k p) -> p k", p=128)

    from contextlib import ExitStack
    with tile.TileContext(nc) as tc, ExitStack() as ctx:
        pools = {
            "const": ctx.enter_context(tc.tile_pool(name="const", bufs=1)),
            "w": ctx.enter_context(tc.tile_pool(name="wpool", bufs=4)),
            "psum": ctx.enter_context(tc.tile_pool(name="pspool", bufs=2, space="PSUM")),
            "psatt": ctx.enter_context(tc.tile_pool(name="psatt", bufs=1, space="PSUM")),
            "pssc": ctx.enter_context(tc.tile_pool(name="pssc", bufs=2, space="PSUM")),
            "lntmp": ctx.enter_context(tc.tile_pool(name="lntmp", bufs=1)),
            "sq": ctx.enter_context(tc.tile_pool(name="sqpool", bufs=2)),
            "o2p": ctx.enter_context(tc.tile_pool(name="o2pool", bufs=1)),
            "acts": ctx.enter_context(tc.tile_pool(name="acts", bufs=1)),
            "qt": ctx.enter_context(tc.tile_pool(name="qt", bufs=2)),
            "kt": ctx.enter_context(tc.tile_pool(name="kt", bufs=2)),
            "vaug": ctx.enter_context(tc.tile_pool(name="vaug", bufs=1)),
            "nh": ctx.enter_context(tc.tile_pool(name="nh", bufs=1)),
            "exp": ctx.enter_context(tc.tile_pool(name="exp", bufs=3)),
            "atmp": ctx.enter_context(tc.tile_pool(name="atmp", bufs=1)),
            "amask": ctx.enter_context(tc.tile_pool(name="amask", bufs=1)),
            "hpool": ctx.enter_context(tc.tile_pool(name="hpool", bufs=1)),
            "o3p": ctx.enter_context(tc.tile_pool(name="o3pool", bufs=1)),
        }
        const = pools["const"]
        acts = pools["acts"]

        tick_sb = const.tile([1, 1], F32, tag="tick", name="tick_sb")
        nc.sync.dma_start(out=tick_sb, in_=tick[:, :])
        ones_bf = const.tile([128, 128], BF16, tag="ones_bf", name="ones_bf")
        nc.vector.memset(ones_bf, 1.0)
        ones_f = const.tile([1, 128], F32, tag="ones_f", name="ones_f")
        nc.vector.memset(ones_f, 1.0)
        ones_row = ones_f.bitcast(F32R)[0:1, :]
        epsc = const.tile([128, 1], F32, tag="epsc", name="epsc")
        nc.vector.memset(epsc, EPS)
        pools["epsc"] = epsc
        bias_cols = {}
        for nm in ("bq1", "bk1", "bo1", "bq2", "bk2", "bo2",
                   "b_ff2", "g1", "be1", "g2", "be2", "g3", "be3"):
            t = const.tile([128, NK], F32, tag=f"col_{nm}", name=f"col_{nm}")
            nc.sync.dma_start(out=t, in_=vcol(nm))
            bias_cols[nm] = t
        bff1c = const.tile([128, NM2], F32, tag="col_bff1", name="col_bff1")
        nc.sync.dma_start(out=bff1c, in_=vcol("b_ff1", DFF))
        mask_sb = pools["amask"].tile([128, NK, 256], BF16, tag="mask", name="mask")
        nc.sync.dma_start(out=mask_sb, in_=maskst[:, :, :])

        bv_aps = {i: vpack[_VOFF[f"bv{i}"]:_VOFF[f"bv{i}"] + D].rearrange(
            "(a c) -> a c", a=1).bitcast(F32R) for i in (1, 2)}
        wff1_h = whalves("w_ff1", NM2)
        wff2_v = ff2_views()

        for _rep in range(reps):
            o2 = [pools["o2p"].tile([128, R], BF16, tag=f"o2_{m}", name=f"o2_{m}")
                  for m in range(NK)]

            # ================= attention =================
            xt_all = acts.tile([128, NK, S], BF16, tag="kv", name="xt_all")
            nc.sync.dma_start(out=xt_all, in_=xt.rearrange("(k p) s -> p k s", p=128))
            xq_all = acts.tile([128, NK, R], BF16, tag="xq", name="xq_all")
            nc.sync.dma_start(out=xq_all, in_=xq.rearrange("(k p) s -> p k s", p=128))
            xt_sb = [xt_all[:, k, :] for k in range(NK)]
            xq_sb = [xq_all[:, k, :] for k in range(NK)]

            y1 = [acts.tile([128, R], BF16, tag=f"y_{m}", name=f"y1_{m}")
                  for m in range(NK)]
            _attention(nc, pools, xq_sb, xt_sb, xq_sb,
                       whalves("wq1", NK), whalves("wk1", NK),
                       whalves("wv1", NK), whalves("wo1", NK),
                       bias_cols["bq1"], bias_cols["bk1"],
                       bv_aps[1], bias_cols["bo1"], mask_sb, y1, ones_row, "a1")
            enc_all = acts.tile([128, NK, S], BF16, tag="env", name="enc_all")
            nc.sync.dma_start(out=enc_all, in_=enc.rearrange("(k p) s -> p k s", p=128))
            enc_sb = [enc_all[:, k, :] for k in range(NK)]
            # o1 reuses the xq slot (xq is dead once y1 is written)
            o1_all = acts.tile([128, NK, R], BF16, tag="xq", name="o1_all")
            o1 = [o1_all[:, m, :] for m in range(NK)]
            _ln(nc, pools, y1, bias_cols["g1"], bias_cols["be1"], o1, ones_bf)

            y2 = [acts.tile([128, R], BF16, tag=f"y_{m}", name=f"y2_{m}")
                  for m in range(NK)]
            _attention(nc, pools, o1, enc_sb, o1,
                       whalves("wq2", NK), whalves("wk2", NK),
                       whalves("wv2", NK), whalves("wo2", NK),
                       bias_cols["bq2"], bias_cols["bk2"],
                       bv_aps[2], bias_cols["bo2"], None, y2, ones_row, "a2")
            _ln(nc, pools, y2, bias_cols["g2"], bias_cols["be2"], o2, ones_bf)

            # ================= FFN =================
            h = []
            wt = None
            for m in range(NM2):
                if m % 4 == 0:
                    wt = pools["w"].tile([128, 4, NK, 128], BF16, tag="wq",
                                         name=f"wff1_{m // 4}")
                    nc.sync.dma_start(out=wt, in_=wff1_h[m // 4])
                psh = pools["psum"].tile([128, R], F32, tag="ps_gen", name="ps_h")
                for k in range(NK):
                    nc.tensor.matmul(psh, wt[:, m % 4, k, :], o2[k],
                                     start=(k == 0), stop=(k == NK - 1))
                h_m = pools["hpool"].tile([128, R], BF16, tag=f"h_{m}", name=f"h_{m}")
                nc.scalar.activation(h_m, psh, AF.Relu, bias=bff1c[:, m:m + 1])
                h.append(h_m)

            y3 = [acts.tile([128, R], BF16, tag=f"y_{m}", name=f"y3_{m}")
                  for m in range(NK)]
            for m in range(NK):
                psf = pools["psum"].tile([128, R], F32, tag="ps_gen", name="ps_f")
                wt = pools["w"].tile([128, 4, NK, 128], BF16, tag="wq",
                                     name=f"wff2_{m}")
                nc.sync.dma_start(out=wt, in_=wff2_v[m])
                for q in range(4):
                    for k in range(NK):
                        nc.tensor.matmul(psf, wt[:, q, k, :], h[q * NK + k],
                                         start=(q == 0 and k == 0),
                                         stop=(q == 3 and k == NK - 1))
                nc.vector.scalar_tensor_tensor(y3[m], psf, bias_cols["b_ff2"][:, m:m + 1],
                                               o2[m], ALU.add, ALU.add)
            o3 = pools["o3p"].tile([128, NK, R], BF16, tag="o3", name="o3")
            o3_tiles = [o3[:, m, :] for m in range(NK)]
            _ln(nc, pools, y3, bias_cols["g3"], bias_cols["be3"], o3_tiles, ones_bf)
            nc.sync.dma_start(out=out_t.rearrange("(m p) r -> p m r", p=128), in_=o3)

    nc.compile()
    return nc


def _get_nc(reps=1):
    if reps not in _NC_CACHE:
        _NC_CACHE[reps] = build_nc(reps)
    return _NC_CACHE[reps]


def _pack_weights(inputs):
    bf = ml_dtypes.bfloat16
    wpack = np.empty((WPACK_N,), dtype=bf)

    def put(nm, arr):
        a = arr.reshape(-1)
        wpack[_WOFF[nm]:_WOFF[nm] + a.size] = a

    for nm in ("wq1", "wk1", "wv1", "wo1", "wq2", "wk2", "wv2", "wo2"):
        w = np.asarray(inputs[nm], dtype=np.float32).astype(bf)
        # [k*128+p, m*128+c] -> [m, p, k, c]
        put(nm, w.reshape(NK, 128, NK, 128).transpose(2, 1, 0, 3))
    w = np.asarray(inputs["w_ff1"], dtype=np.float32).astype(bf)
    put("w_ff1", w.reshape(NK, 128, NM2, 128).transpose(2, 1, 0, 3))
    w = np.asarray(inputs["w_ff2"], dtype=np.float32).astype(bf)
    # [q*8*128 + k*128 + p, m*128+c] -> [m, q, p, k, c]
    put("w_ff2", w.reshape(DFF // D, NK, 128, NK, 128).transpose(3, 0, 2, 1, 4))

    vpack = np.empty((VPACK_N,), dtype=np.float32)
    for nm in ("bq1", "bk1", "bo1", "bq2", "bk2", "bo2", "bv1", "bv2",
               "b_ff1", "b_ff2", "g1", "be1", "g2", "be2", "g3", "be3"):
        a = np.asarray(inputs[nm], dtype=np.float32).reshape(-1)
        vpack[_VOFF[nm]:_VOFF[nm] + a.size] = a
    return wpack, vpack


def _make_in_maps(inputs):
    full_k = np.arange(S)
    wpack, vpack = _pack_weights(inputs)
    in_maps = []
    metas = []
    for c in range(8):
        b, half = c // 2, c % 2
        qidx = np.concatenate([np.arange(128) + 128 * blk for blk in BLOCKS[half]])
        xt_b = np.ascontiguousarray(np.asarray(inputs["inputs"][b]).T.astype(ml_dtypes.bfloat16))
        enc_b = np.ascontiguousarray(np.asarray(inputs["enc_outputs"][b]).T.astype(ml_dtypes.bfloat16))
        xq_b = np.ascontiguousarray(xt_b[:, qidx])
        mask = np.where(full_k[:, None] <= qidx[None, :], 1.0, 0.0).astype(ml_dtypes.bfloat16)
        mask8 = np.stack([mask[s * 128:(s + 1) * 128, 128 * (s // 2):128 * (s // 2) + 128]
                          for s in range(NK)])           # [s, p, c]
        mask8 = np.concatenate([mask8, mask8], axis=-1)  # duplicate per head
        mask8 = np.ascontiguousarray(mask8.transpose(1, 0, 2))   # [p, s, 2*c]
        m = {"xt": xt_b, "xq": xq_b, "enc": enc_b,
             "maskst": mask8,
             "wpack": wpack, "vpack": vpack,
             "tick": np.zeros((1, 1), np.float32)}
        in_maps.append(m)
        metas.append((b, qidx))
    return in_maps, metas


def kernel(**inputs):
    nc = _get_nc()
    in_maps, metas = _make_in_maps(inputs)
    res = run_bass_kernel_spmd(nc, in_maps, core_ids=list(range(8)))
    out = np.zeros((B, S, D), dtype=np.float32)
    for c, (b, qidx) in enumerate(metas):
        out[b, qidx, :] = res.results[c]["out_t"].astype(np.float32).T
    return out



# revision 22
# speedup vs baseline: 1.4544x; 1.4544x over previous
"""Fused transformer decoder layer (self-attn + cross-attn + FFN, 3 LayerNorms)
for Trainium2, SPMD across 8 NeuronCores.

Sharding: 2 cores per batch element (B=4). Each core owns 512 query rows of
its batch element, picked as four 128-row blocks interleaved so the causal
self-attention work is balanced across the pair ({0,3,4,7} / {1,2,5,6}).
K/V projections are computed redundantly per core (no collectives needed).

On-device layout: activations are kept feature-major ("transposed", [D, rows])
so every linear layer uses the weight matrices exactly as stored:
    out^T [Dout, r] = matmul(lhsT=W[D, Dout]-tile, rhs=act^T[D, r]-tile).
Attention scores are computed transposed (scores^T[k, q] = K·Q^T per head);
softmax denominators are harvested by augmenting V with 64 all-ones columns,
which lands the per-query sums partition-replicated next to the attnV output.
The causal mask is applied multiplicatively post-exp on the GpSimd engine.

v2 scheduling notes (vs the original baseline):
  - big input/weight DMA issues are emitted first (dma_start issue on the
    SP sequencer costs ~1us each; small const DMAs used to delay the input
    transfers by ~20us),
  - all small fp32 params ride in ONE [128, 152] DMA,
  - V bias is applied by a broadcast-row add fused into the PSUM->SBUF copy
    (replaces 64 K=1 matmuls),
  - V/K production runs one pair-block ahead of the Q/score/attnV pipeline,
    so tensor work exists to cover each LayerNorm tail and weight-DMA latency,
  - LayerNorm statistics are interleaved into the producing projection loop,
  - softmax / LN reciprocals use the fast approximate DVE reciprocal,
  - the final output is DMA'd per 128-row block as LN3 produces it.

Everything on the matmul path is bf16 (weights, activations, probs); LN
stats and softmax normalization run in fp32. Weights are host-repacked into
the on-device tile layout ([m-block, partition, k-block, col] order) inside
one flat DRAM tensor, so every weight DMA moves 1 MB with >=2 KB contiguous
per partition line. Small fp32 params ride in a second flat tensor.
Output is bf16.

build_nc(reps) emits the layer `reps` times in one program (one NEFF) so
dispatch overhead can be amortized over genuine back-to-back executions;
the grading entry point uses reps=1.
"""

import numpy as np
import ml_dtypes

import concourse.bacc as bacc
import concourse.bass as bass
import concourse.mybir as mybir
import concourse.tile as tile
from concourse.bass_utils import run_bass_kernel_spmd

F32 = mybir.dt.float32
F32R = mybir.dt.float32r
BF16 = mybir.dt.bfloat16
AF = mybir.ActivationFunctionType
ALU = mybir.AluOpType

B, S, D, DFF, H = 4, 1024, 1024, 4096, 16
R = 512                   # query rows per core
NK = D // 128             # 8 k-tiles over D
NP = H // 2               # 8 head pairs
NM2 = DFF // 128          # 32 m-tiles over DFF
EPS = 1e-3
BLOCKS = [[0, 3, 4, 7], [1, 2, 5, 6]]   # 128-row q-blocks per half-core

# element offsets into the flat bf16 weight pack
_WOFF = {}
_off = 0
for _nm, _sz in (("wq1", D * D), ("wk1", D * D), ("wv1", D * D), ("wo1", D * D),
                 ("wq2", D * D), ("wk2", D * D), ("wv2", D * D), ("wo2", D * D),
                 ("w_ff1", D * DFF), ("w_ff2", DFF * D)):
    _WOFF[_nm] = _off
    _off += _sz
WPACK_N = _off

# element offsets into the flat f32 param pack
_VOFF = {}
_off = 0
for _nm, _sz in (("bq1", D), ("bk1", D), ("bo1", D),
                 ("bq2", D), ("bk2", D), ("bo2", D),
                 ("bv1", D), ("bv2", D),
                 ("b_ff1", DFF), ("b_ff2", D),
                 ("g1", D), ("be1", D), ("g2", D), ("be2", D),
                 ("g3", D), ("be3", D)):
    _VOFF[_nm] = _off
    _off += _sz
VPACK_N = _off

_NC_CACHE = {}
DEBUG_TAPS = False


def _ln_stats(nc, pools, psln, m, y_m, ones_bf):
    """Accumulate sum / sq-sum of one [128, R] y tile into psln."""
    nc.tensor.matmul(psln[:, 0, :], ones_bf, y_m,
                     start=(m == 0), stop=(m == NK - 1))
    sq = pools["sq"].tile([128, R], BF16, tag="ln_sqt", name="ln_sqt")
    nc.scalar.activation(sq, y_m, AF.Square)
    nc.tensor.matmul(psln[:, 1, :], ones_bf, sq,
                     start=(m == 0), stop=(m == NK - 1))


def _ln_finish(nc, pools, psln, y, gcol, bcol, out_tiles):
    """Finish LayerNorm from accumulated stats; write out_tiles per m."""
    tmp = pools["lntmp"]
    pssum, pssq = psln[:, 0, :], psln[:, 1, :]
    mean = tmp.tile([128, R], F32, tag="ln_mean", name="ln_mean")
    nc.vector.tensor_scalar_mul(mean, pssum, 1.0 / D)
    rv = tmp.tile([128, R], F32, tag="ln_sc", name="ln_rv")
    nc.vector.tensor_scalar_mul(rv, pssq, 1.0 / D)      # E[x^2]
    msq = tmp.tile([128, R], F32, tag="ln_t0", name="ln_msq")
    nc.vector.tensor_mul(msq, mean, mean)
    nc.vector.tensor_sub(rv, rv, msq)                   # var
    nc.scalar.activation(rv, rv, AF.Sqrt, bias=pools["epsc"][:, 0:1])
    nc.vector.reciprocal(rv, rv)                        # rstd (broadcast)
    nc.vector.tensor_mul(mean, mean, rv)                # mean*rstd (broadcast)
    for m in range(NK):
        t = tmp.tile([128, R], F32, tag=f"ln_t{m % 3}", name="ln_t")
        nc.vector.tensor_mul(t, y[m], rv)
        nc.vector.tensor_sub(t, t, mean)
        nc.vector.tensor_scalar(out_tiles[m], t, gcol[:, m:m + 1], bcol[:, m:m + 1],
                                ALU.mult, ALU.add)


def _issue_w0(nc, pools, wv_h, wk_h, wq_h, label):
    """DMA the first weight half of an attention; call early to prefetch."""
    w0 = {}
    for nm, h in (("wv", wv_h), ("wk", wk_h), ("wq", wq_h)):
        t = pools["w"].tile([128, 4, NK, 128], BF16, tag="wq",
                            name=f"{nm}0_{label}")
        nc.sync.dma_start(out=t, in_=h[0])
        w0[nm] = t
    return w0


def _attention(nc, pools, q_src, kv_src, resid, w0, wq_h, wk_h, wv_h, wo_h,
               bqc, bkc, bv_row, boc, mask_sb, y_out, ones_row, ones_bf,
               label, prefetch_cb=None):
    """One multi-head attention block + residual; writes pre-LN y_out tiles
    and accumulates LN stats into a psln tile (returned) as each y tile is
    produced.

    w0: pre-issued {wv,wk,wq} tiles for m-blocks 0..3 (from _issue_w0).
    wq_h/wk_h/wv_h/wo_h: per-projection "half" APs — lists of 2 DRAM views
    [128(p), 4(m), NK(k), 128(c)] covering m-blocks {4i..4i+3}.
    bv_row: [1, D] f32r SBUF row of the V bias.
    """
    wpool, ps, tmp = pools["w"], pools["psum"], pools["atmp"]
    causal = mask_sb is not None

    # persistent V_aug; 4 pair slots so V production can run one pair-block
    # ahead of consumption. layout [128, pair%4, s(NK), 256] with per-s
    # columns [v_h0(64) | ones(128) | v_h1(64)]
    vaug = pools["vaug"].tile([128, 4, NK, 256], BF16, tag="vaug",
                              name=f"vaug_{label}")
    nc.vector.memset(vaug[:, :, :, 64:192], 1.0)

    # V bias broadcast to all partitions: bvrep[p, c] = bv[c]
    bvrep = pools["bvr"].tile([128, D], BF16, tag="bvrep", name=f"bvr_{label}")
    for half in range(2):
        psb = ps.tile([128, R], F32, tag="ps_gen", name="ps_bv")
        nc.tensor.matmul(psb, ones_row, bv_row[:, half * 512:(half + 1) * 512],
                         start=True, stop=True)
        nc.vector.tensor_copy(bvrep[:, half * 512:(half + 1) * 512], psb)

    state = {"wv0": w0["wv"], "wk0": w0["wk"], "wq0": w0["wq"]}

    def emit_vk(jj):
        """V for pairs {2jj, 2jj+1} into vaug, K^T tiles for those pairs."""
        i, jl = jj // 2, jj % 2
        if jj == 1:
            # prefetch the second weight half one pair-block early
            for nm, hh in (("wv1", wv_h), ("wk1", wk_h), ("wq1", wq_h)):
                t = wpool.tile([128, 4, NK, 128], BF16, tag="wq",
                               name=f"{nm}_{label}")
                nc.sync.dma_start(out=t, in_=hh[1])
                state[nm] = t
        wv_cur, wk_cur = state[f"wv{i}"], state[f"wk{i}"]
        # ---- V for the two pairs produced together (moving dim 256) ----
        bvv = bvrep[:, jj * 256:(jj + 1) * 256].rearrange(
            "p (i a c) -> p i a c", a=2, c=64)
        for s in range(NK):
            psv = ps.tile([128, 256], F32, tag="ps_gen", name="ps_v")
            for k in range(NK):
                nc.tensor.matmul(psv, kv_src[k][:, s * 128:(s + 1) * 128],
                                 wv_cur[:, 2 * jl:2 * jl + 2, k, :],
                                 start=(k == 0), stop=(k == NK - 1))
            # single fused copy+bias drops both pairs' V around the ones
            sl = (2 * jj) % 4
            dst = vaug[:, sl:sl + 2, s, :].rearrange(
                "p i (a c) -> p i a c", c=64)[:, :, 0:4:3, :]
            src = psv.rearrange("p (i a c) -> p i a c", a=2, c=64)
            nc.vector.tensor_add(dst, src, bvv)
        # ---- K^T for the two pairs ----
        for p in (2 * jj, 2 * jj + 1):
            pl = p % 4
            kt_p = pools["kt"].tile([128, S], BF16, tag="kt",
                                    name=f"kt_{label}_{p}")
            for half in range(2):
                psk = ps.tile([128, R], F32, tag="ps_gen", name="ps_k")
                for k in range(NK):
                    nc.tensor.matmul(psk, wk_cur[:, pl, k, :],
                                     kv_src[k][:, half * 512:(half + 1) * 512],
                                     start=(k == 0), stop=(k == NK - 1))
                nc.scalar.activation(kt_p[:, half * 512:(half + 1) * 512], psk,
                                     AF.Identity, bias=bkc[:, p:p + 1])
            state[f"kt{p}"] = kt_p

    emit_vk(0)
    nh = []
    for p in range(NP):
        if p % 2 == 0 and p // 2 + 1 <= 3:
            emit_vk(p // 2 + 1)        # keep V/K one pair-block ahead
        pl = p % 4
        kt_p = state.pop(f"kt{p}")

        # ---- Q^T for this pair ----
        psq = ps.tile([128, R], F32, tag="ps_gen", name="ps_q")
        for k in range(NK):
            nc.tensor.matmul(psq, state[f"wq{p // 4}"][:, pl, k, :], q_src[k],
                             start=(k == 0), stop=(k == NK - 1))
        qt_p = pools["qt"].tile([128, R], BF16, tag="qt", name=f"qt_{label}_{p}")
        nc.scalar.activation(qt_p, psq, AF.Identity, bias=bqc[:, p:p + 1])

        # ---- scores^T, exp, attnV (softmax sums ride along in V_aug ones) --
        # Both heads share one 2-bank PSUM tile; exp runs once over both.
        va = vaug[:, p % 4]
        psap = pools["psatt"].tile([128, 2, R], F32, tag="ps_att", name="ps_att")
        for s in range(NK):
            c0 = 128 * (s // 2) if causal else 0
            sscp = pools["pssc"].tile([128, 2, R], F32, tag="ps_sc", name="ps_sc")
            nc.tensor.matmul(sscp[:, 0, c0:R], kt_p[0:64, s * 128:(s + 1) * 128],
                             qt_p[0:64, c0:R], start=True, stop=True)
            nc.tensor.matmul(sscp[:, 1, c0:R], kt_p[64:128, s * 128:(s + 1) * 128],
                             qt_p[64:128, c0:R], start=True, stop=True)
            e = pools["exp"].tile([128, 2, R], BF16, tag="e", name="e")
            nc.scalar.activation(e[:, :, c0:R], sscp[:, :, c0:R], AF.Exp,
                                 scale=0.125)
            if causal:
                # zero the future half of the diagonal q-block (binary mask,
                # duplicated per head) on the otherwise-idle GpSimd engine
                nc.gpsimd.tensor_mul(
                    e[:, :, c0:c0 + 128], e[:, :, c0:c0 + 128],
                    mask_sb[:, s, :].rearrange("p (h c) -> p h c", h=2))
            nc.tensor.matmul(psap[:, 0, c0:R], va[:, s, 0:128], e[:, 0, c0:R],
                             start=(s == 0), stop=(s == NK - 1),
                             skip_group_check=True)
            nc.tensor.matmul(psap[:, 1, c0:R], va[:, s, 128:256], e[:, 1, c0:R],
                             start=(s == 0), stop=(s == NK - 1),
                             skip_group_check=True)

        # evacuate the attnV accumulator promptly so the next pair's attnV can
        # reuse the psatt banks without waiting for the normalize chain
        pcp = pools["sq"].tile([128, 2, R], F32, tag="pcp", name="pcp")
        nc.vector.tensor_copy(pcp, psap)

        # normalize: head0 out rows 0:64 / sums 64:128; head1 sums 0:64 / out 64:128.
        # Bounce both heads' sums into out-aligned partitions, then a single
        # reciprocal covers both heads (DVE reciprocal cost is per-column).
        nh_p = pools["nh"].tile([128, R], BF16, tag=f"nh_{p}", name=f"nh_{label}_{p}")
        sums = tmp.tile([128, R], F32, tag="inv", name="sums")
        nc.sync.dma_start(out=sums[0:64, :], in_=pcp[64:128, 0, :])
        nc.sync.dma_start(out=sums[64:128, :], in_=pcp[0:64, 1, :])
        inv = tmp.tile([128, R], F32, tag="invs", name="inv")
        nc.vector.reciprocal(inv, sums)
        nc.gpsimd.tensor_mul(nh_p[0:64, :], pcp[0:64, 0, :], inv[0:64, :])
        nc.gpsimd.tensor_mul(nh_p[64:128, :], pcp[64:128, 1, :], inv[64:128, :])
        nh.append(nh_p)
        if p == NP - 2:
            state["wo0"] = wpool.tile([128, 4, NK, 128], BF16, tag="wq",
                                      name=f"woc_{label}_0")
            nc.sync.dma_start(out=state["wo0"], in_=wo_h[0])
        if p == NP - 1:
            state["wo1"] = wpool.tile([128, 4, NK, 128], BF16, tag="wq",
                                      name=f"woc_{label}_1")
            nc.sync.dma_start(out=state["wo1"], in_=wo_h[1])

    if prefetch_cb is not None:
        prefetch_cb()   # issue the next phase's first weight DMAs now

    # ---- output projection + bias + residual (+ LN stats per tile) ----
    psln = pools["psatt"].tile([128, 2, R], F32, tag="ps_att",
                               name=f"ln_ps_{label}")
    for m in range(NK):
        wo_cur = state["wo0"] if m < 4 else state["wo1"]
        pso = ps.tile([128, R], F32, tag="ps_gen", name="ps_o")
        for p in range(NP):
            nc.tensor.matmul(pso, wo_cur[:, m % 4, p, :], nh[p],
                             start=(p == 0), stop=(p == NP - 1))
        nc.vector.scalar_tensor_tensor(y_out[m], pso, boc[:, m:m + 1],
                                       resid[m], ALU.add, ALU.add)
        _ln_stats(nc, pools, psln, m, y_out[m], ones_bf)
    return psln


def build_nc(reps=1):
    nc = bacc.Bacc("TRN2", target_bir_lowering=False, debug=False)

    xt = nc.dram_tensor("xt", [D, S], BF16, kind="ExternalInput")
    xq = nc.dram_tensor("xq", [D, R], BF16, kind="ExternalInput")
    enc = nc.dram_tensor("enc", [D, S], BF16, kind="ExternalInput")
    maskst = nc.dram_tensor("maskst", [128, NK, 256], BF16, kind="ExternalInput")
    wpack = nc.dram_tensor("wpack", [WPACK_N], BF16, kind="ExternalInput")
    vpack = nc.dram_tensor("vpack", [VPACK_N], F32, kind="ExternalInput")
    out_t = nc.dram_tensor("out_t", [D, R], BF16, kind="ExternalOutput")

    def whalves(nm, nm_blocks):
        # host layout [m, p, k, c]; -> list of [128(p), 4(m), NK(k), 128(c)]
        full = wpack[_WOFF[nm]:_WOFF[nm] + nm_blocks * D * 128].rearrange(
            "(m p k c) -> p m k c", p=128, k=NK, c=128)
        return [full[:, 4 * i:4 * i + 4] for i in range(nm_blocks // 4)]

    def ff2_views():
        # host layout [m, q, p, k, c]; -> per m: [128, 4(q), NK, 128]
        full = wpack[_WOFF["w_ff2"]:_WOFF["w_ff2"] + DFF * D].rearrange(
            "(m q p k c) -> p m q k c", q=DFF // D, p=128, k=NK, c=128)
        return [full[:, m] for m in range(NK)]

    from contextlib import ExitStack
    with tile.TileContext(nc) as tc, ExitStack() as ctx:
        pools = {
            "const": ctx.enter_context(tc.tile_pool(name="const", bufs=1)),
            "w": ctx.enter_context(tc.tile_pool(name="wpool", bufs=4)),
            "psum": ctx.enter_context(tc.tile_pool(name="pspool", bufs=2, space="PSUM")),
            "psatt": ctx.enter_context(tc.tile_pool(name="psatt", bufs=1, space="PSUM")),
            "pssc": ctx.enter_context(tc.tile_pool(name="pssc", bufs=2, space="PSUM")),
            "lntmp": ctx.enter_context(tc.tile_pool(name="lntmp", bufs=1)),
            "sq": ctx.enter_context(tc.tile_pool(name="sqpool", bufs=2)),
            "o2p": ctx.enter_context(tc.tile_pool(name="o2pool", bufs=1)),
            "acts": ctx.enter_context(tc.tile_pool(name="acts", bufs=1)),
            "qt": ctx.enter_context(tc.tile_pool(name="qt", bufs=2)),
            "kt": ctx.enter_context(tc.tile_pool(name="kt", bufs=4)),
            "vaug": ctx.enter_context(tc.tile_pool(name="vaug", bufs=1)),
            "bvr": ctx.enter_context(tc.tile_pool(name="bvr", bufs=1)),
            "nh": ctx.enter_context(tc.tile_pool(name="nh", bufs=1)),
            "exp": ctx.enter_context(tc.tile_pool(name="exp", bufs=3)),
            "atmp": ctx.enter_context(tc.tile_pool(name="atmp", bufs=2)),
            "amask": ctx.enter_context(tc.tile_pool(name="amask", bufs=1)),
            "hpool": ctx.enter_context(tc.tile_pool(name="hpool", bufs=1)),
        }
        const = pools["const"]
        acts = pools["acts"]

        wff1_h = whalves("w_ff1", NM2)
        wff2_v = ff2_views()

        for _rep in range(reps):
            # ---- big DMAs first: dma_start issue is ~1us each on SP ----
            xt_all = acts.tile([128, NK, S], BF16, tag="kv", name="xt_all")
            nc.sync.dma_start(out=xt_all, in_=xt.rearrange("(k p) s -> p k s", p=128))
            xq_all = acts.tile([128, NK, R], BF16, tag="xq", name="xq_all")
            nc.sync.dma_start(out=xq_all, in_=xq.rearrange("(k p) s -> p k s", p=128))
            wq1_h, wk1_h, wv1_h = whalves("wq1", NK), whalves("wk1", NK), whalves("wv1", NK)
            w0_a1 = _issue_w0(nc, pools, wv1_h, wk1_h, wq1_h, "a1")

            if _rep == 0:
                # ---- all small fp32 params in one DMA ----
                vall = const.tile([128, VPACK_N // 128], F32, tag="vall", name="vall")
                nc.sync.dma_start(out=vall, in_=vpack.rearrange("(n p) -> p n", p=128))
                bias_cols = {nm: vall[:, _VOFF[nm] // 128:_VOFF[nm] // 128 + NK]
                             for nm in ("bq1", "bk1", "bo1", "bq2", "bk2", "bo2",
                                        "b_ff2", "g1", "be1", "g2", "be2",
                                        "g3", "be3")}
                bff1c = vall[:, _VOFF["b_ff1"] // 128:_VOFF["b_ff1"] // 128 + NM2]
                # V biases as rows (for partition-broadcast via K=1 matmul);
                # the f32 -> f32r rounding happens inside the DMA
                bvrow = const.tile([1, 2 * D], F32R, tag="bvrow", name="bvrow")
                nc.sync.dma_start(
                    out=bvrow,
                    in_=vpack[_VOFF["bv1"]:_VOFF["bv1"] + 2 * D].rearrange(
                        "(a c) -> a c", a=1).bitcast(F32R))
                bv_rows = {1: bvrow[0:1, 0:D], 2: bvrow[0:1, D: 2 * D]}
                mask_sb = pools["amask"].tile([128, NK, 256], BF16, tag="mask",
                                              name="mask")
                nc.sync.dma_start(out=mask_sb, in_=maskst[:, :, :])
                ones_bf = const.tile([128, 128], BF16, tag="ones_bf", name="ones_bf")
                nc.vector.memset(ones_bf, 1.0)
                ones_f = const.tile([1, 128], F32, tag="ones_f", name="ones_f")
                nc.vector.memset(ones_f, 1.0)
                ones_row = ones_f.bitcast(F32R)[0:1, :]
                epsc = const.tile([128, 1], F32, tag="epsc", name="epsc")
                nc.vector.memset(epsc, EPS)
                pools["epsc"] = epsc

            enc_all = acts.tile([128, NK, S], BF16, tag="env", name="enc_all")
            nc.sync.dma_start(out=enc_all, in_=enc.rearrange("(k p) s -> p k s", p=128))

            xt_sb = [xt_all[:, k, :] for k in range(NK)]
            xq_sb = [xq_all[:, k, :] for k in range(NK)]
            enc_sb = [enc_all[:, k, :] for k in range(NK)]

            o2 = [pools["o2p"].tile([128, R], BF16, tag=f"o2_{m}", name=f"o2_{m}")
                  for m in range(NK)]

            # ================= attention =================
            wq2_h, wk2_h, wv2_h = (whalves("wq2", NK), whalves("wk2", NK),
                                   whalves("wv2", NK))
            w0_a2 = {}

            def prefetch_a2():
                w0_a2.update(_issue_w0(nc, pools, wv2_h, wk2_h, wq2_h, "a2"))

            y1 = [acts.tile([128, R], BF16, tag=f"y_{m}", name=f"y1_{m}")
                  for m in range(NK)]
            psln1 = _attention(nc, pools, xq_sb, xt_sb, xq_sb, w0_a1,
                               wq1_h, wk1_h, wv1_h, whalves("wo1", NK),
                               bias_cols["bq1"], bias_cols["bk1"],
                               bv_rows[1], bias_cols["bo1"], mask_sb, y1,
                               ones_row, ones_bf, "a1", prefetch_cb=prefetch_a2)
            # o1 reuses the xq slot (xq is dead once y1 is written)
            o1_all = acts.tile([128, NK, R], BF16, tag="xq", name="o1_all")
            o1 = [o1_all[:, m, :] for m in range(NK)]
            _ln_finish(nc, pools, psln1, y1, bias_cols["g1"], bias_cols["be1"], o1)
            if DEBUG_TAPS:
                dbg_y1 = nc.dram_tensor("dbg_y1", [128, NK, R], BF16,
                                        kind="ExternalOutput")
                for m in range(NK):
                    nc.sync.dma_start(out=dbg_y1[:, m, :], in_=y1[m])
                dbg_o1 = nc.dram_tensor("dbg_o1", [128, NK, R], BF16,
                                        kind="ExternalOutput")
                nc.sync.dma_start(out=dbg_o1[:, :, :], in_=o1_all)

            wt_next = None

            def prefetch_ffn():
                nonlocal wt_next
                wt_next = pools["w"].tile([128, 4, NK, 128], BF16, tag="wq",
                                          name="wff1_0")
                nc.sync.dma_start(out=wt_next, in_=wff1_h[0])

            y2 = [acts.tile([128, R], BF16, tag=f"y_{m}", name=f"y2_{m}")
                  for m in range(NK)]
            psln2 = _attention(nc, pools, o1, enc_sb, o1, w0_a2,
                               wq2_h, wk2_h, wv2_h, whalves("wo2", NK),
                               bias_cols["bq2"], bias_cols["bk2"],
                               bv_rows[2], bias_cols["bo2"], None, y2,
                               ones_row, ones_bf, "a2", prefetch_cb=prefetch_ffn)
            _ln_finish(nc, pools, psln2, y2, bias_cols["g2"], bias_cols["be2"], o2)
            if DEBUG_TAPS:
                dbg_y2 = nc.dram_tensor("dbg_y2", [128, NK, R], BF16,
                                        kind="ExternalOutput")
                for m in range(NK):
                    nc.sync.dma_start(out=dbg_y2[:, m, :], in_=y2[m])
                dbg_o2 = nc.dram_tensor("dbg_o2", [128, NK, R], BF16,
                                        kind="ExternalOutput")
                for m in range(NK):
                    nc.sync.dma_start(out=dbg_o2[:, m, :], in_=o2[m])

            # ================= FFN =================
            h = []
            wt = None
            for m in range(NM2):
                if m % 4 == 0:
                    wt = wt_next
                    if m // 4 + 1 < NM2 // 4:
                        wt_next = pools["w"].tile([128, 4, NK, 128], BF16,
                                                  tag="wq", name=f"wff1_{m // 4 + 1}")
                        nc.sync.dma_start(out=wt_next, in_=wff1_h[m // 4 + 1])
                    elif m // 4 + 1 == NM2 // 4:
                        wt_next = pools["w"].tile([128, 4, NK, 128], BF16,
                                                  tag="wq", name="wff2_0")
                        nc.sync.dma_start(out=wt_next, in_=wff2_v[0])
                psh = pools["psum"].tile([128, R], F32, tag="ps_gen", name="ps_h")
                for k in range(NK):
                    nc.tensor.matmul(psh, wt[:, m % 4, k, :], o2[k],
                                     start=(k == 0), stop=(k == NK - 1))
                h_m = pools["hpool"].tile([128, R], BF16, tag=f"h_{m}", name=f"h_{m}")
                nc.scalar.activation(h_m, psh, AF.Relu, bias=bff1c[:, m:m + 1])
                h.append(h_m)

            y3 = [acts.tile([128, R], BF16, tag=f"y_{m}", name=f"y3_{m}")
                  for m in range(NK)]
            psln3 = pools["psatt"].tile([128, 2, R], F32, tag="ps_att", name="ln3_ps")
            for m in range(NK):
                psf = pools["psum"].tile([128, R], F32, tag="ps_gen", name="ps_f")
                wt = wt_next
                if m + 1 < NK:
                    wt_next = pools["w"].tile([128, 4, NK, 128], BF16, tag="wq",
                                              name=f"wff2_{m + 1}")
                    nc.sync.dma_start(out=wt_next, in_=wff2_v[m + 1])
                for q in range(4):
                    for k in range(NK):
                        nc.tensor.matmul(psf, wt[:, q, k, :], h[q * NK + k],
                                         start=(q == 0 and k == 0),
                                         stop=(q == 3 and k == NK - 1))
                nc.vector.scalar_tensor_tensor(y3[m], psf, bias_cols["b_ff2"][:, m:m + 1],
                                               o2[m], ALU.add, ALU.add)
                _ln_stats(nc, pools, psln3, m, y3[m], ones_bf)
            # o3 reuses the xq slot (o1 is dead once y2 is written)
            o3_all = acts.tile([128, NK, R], BF16, tag="xq", name="o3_all")
            o3 = [o3_all[:, m, :] for m in range(NK)]
            _ln_finish(nc, pools, psln3, y3, bias_cols["g3"], bias_cols["be3"], o3)
            out_v = out_t.rearrange("(m p) r -> p m r", p=128)
            for m in range(NK):
                nc.sync.dma_start(out=out_v[:, m, :], in_=o3_all[:, m, :])

    nc.compile()
    return nc


def _get_nc(reps=1):
    if reps not in _NC_CACHE:
        _NC_CACHE[reps] = build_nc(reps)
    return _NC_CACHE[reps]


def _pack_weights(inputs):
    bf = ml_dtypes.bfloat16
    wpack = np.empty((WPACK_N,), dtype=bf)

    def put(nm, arr):
        a = arr.reshape(-1)
        wpack[_WOFF[nm]:_WOFF[nm] + a.size] = a

    for nm in ("wq1", "wk1", "wv1", "wo1", "wq2", "wk2", "wv2", "wo2"):
        w = np.asarray(inputs[nm], dtype=np.float32).astype(bf)
        # [k*128+p, m*128+c] -> [m, p, k, c]
        put(nm, w.reshape(NK, 128, NK, 128).transpose(2, 1, 0, 3))
    w = np.asarray(inputs["w_ff1"], dtype=np.float32).astype(bf)
    put("w_ff1", w.reshape(NK, 128, NM2, 128).transpose(2, 1, 0, 3))
    w = np.asarray(inputs["w_ff2"], dtype=np.float32).astype(bf)
    # [q*8*128 + k*128 + p, m*128+c] -> [m, q, p, k, c]
    put("w_ff2", w.reshape(DFF // D, NK, 128, NK, 128).transpose(3, 0, 2, 1, 4))

    vpack = np.empty((VPACK_N,), dtype=np.float32)
    for nm in ("bq1", "bk1", "bo1", "bq2", "bk2", "bo2", "bv1", "bv2",
               "b_ff1", "b_ff2", "g1", "be1", "g2", "be2", "g3", "be3"):
        a = np.asarray(inputs[nm], dtype=np.float32).reshape(-1)
        vpack[_VOFF[nm]:_VOFF[nm] + a.size] = a
    return wpack, vpack


def _make_in_maps(inputs):
    full_k = np.arange(S)
    wpack, vpack = _pack_weights(inputs)
    in_maps = []
    metas = []
    for c in range(8):
        b, half = c // 2, c % 2
        qidx = np.concatenate([np.arange(128) + 128 * blk for blk in BLOCKS[half]])
        xt_b = np.ascontiguousarray(np.asarray(inputs["inputs"][b]).T.astype(ml_dtypes.bfloat16))
        enc_b = np.ascontiguousarray(np.asarray(inputs["enc_outputs"][b]).T.astype(ml_dtypes.bfloat16))
        xq_b = np.ascontiguousarray(xt_b[:, qidx])
        mask = np.where(full_k[:, None] <= qidx[None, :], 1.0, 0.0).astype(ml_dtypes.bfloat16)
        mask8 = np.stack([mask[s * 128:(s + 1) * 128, 128 * (s // 2):128 * (s // 2) + 128]
                          for s in range(NK)])           # [s, p, c]
        mask8 = np.concatenate([mask8, mask8], axis=-1)  # duplicate per head
        mask8 = np.ascontiguousarray(mask8.transpose(1, 0, 2))   # [p, s, 2*c]
        m = {"xt": xt_b, "xq": xq_b, "enc": enc_b,
             "maskst": mask8,
             "wpack": wpack, "vpack": vpack}
        in_maps.append(m)
        metas.append((b, qidx))
    return in_maps, metas


def kernel(**inputs):
    nc = _get_nc()
    in_maps, metas = _make_in_maps(inputs)
    res = run_bass_kernel_spmd(nc, in_maps, core_ids=list(range(8)))
    out = np.zeros((B, S, D), dtype=np.float32)
    for c, (b, qidx) in enumerate(metas):
        out[b, qidx, :] = res.results[c]["out_t"].astype(np.float32).T
    return out


# revision 30
# speedup vs baseline: 1.4827x; 1.0195x over previous
"""Fused transformer decoder layer (self-attn + cross-attn + FFN, 3 LayerNorms)
for Trainium2, SPMD across 8 NeuronCores.

Sharding: 2 cores per batch element (B=4). Each core owns 512 query rows of
its batch element, picked as four 128-row blocks interleaved so the causal
self-attention work is balanced across the pair ({0,3,4,7} / {1,2,5,6}).
K/V projections are computed redundantly per core (no collectives needed).

On-device layout: activations are kept feature-major ("transposed", [D, rows])
so every linear layer uses the weight matrices exactly as stored:
    out^T [Dout, r] = matmul(lhsT=W[D, Dout]-tile, rhs=act^T[D, r]-tile).
Attention scores are computed transposed (scores^T[k, q] = K·Q^T per head);
softmax denominators are harvested by augmenting V with 64 all-ones columns,
which lands the per-query sums partition-replicated next to the attnV output.
The causal mask is applied multiplicatively post-exp on the GpSimd engine.

v2 scheduling notes (vs the original baseline):
  - big input/weight DMA issues are emitted first (dma_start issue on the
    SP sequencer costs ~1us each; small const DMAs used to delay the input
    transfers by ~20us),
  - all small fp32 params ride in ONE [128, 152] DMA,
  - V bias is applied by a broadcast-row add fused into the PSUM->SBUF copy
    (replaces 64 K=1 matmuls),
  - V/K production runs one pair-block ahead of the Q/score/attnV pipeline,
    so tensor work exists to cover each LayerNorm tail and weight-DMA latency,
  - LayerNorm statistics are interleaved into the producing projection loop,
  - softmax / LN reciprocals use the fast approximate DVE reciprocal,
  - the final output is DMA'd per 128-row block as LN3 produces it.

Everything on the matmul path is bf16 (weights, activations, probs); LN
stats and softmax normalization run in fp32. Weights are host-repacked into
the on-device tile layout ([m-block, partition, k-block, col] order) inside
one flat DRAM tensor, so every weight DMA moves 1 MB with >=2 KB contiguous
per partition line. Small fp32 params ride in a second flat tensor.
Output is bf16.

build_nc(reps) emits the layer `reps` times in one program (one NEFF) so
dispatch overhead can be amortized over genuine back-to-back executions;
the grading entry point uses reps=1.
"""

import numpy as np
import ml_dtypes

import concourse.bacc as bacc
import concourse.bass as bass
import concourse.mybir as mybir
import concourse.tile as tile
from concourse.bass_utils import run_bass_kernel_spmd

F32 = mybir.dt.float32
F32R = mybir.dt.float32r
BF16 = mybir.dt.bfloat16
AF = mybir.ActivationFunctionType
ALU = mybir.AluOpType

B, S, D, DFF, H = 4, 1024, 1024, 4096, 16
R = 512                   # query rows per core
NK = D // 128             # 8 k-tiles over D
NP = H // 2               # 8 head pairs
NM2 = DFF // 128          # 32 m-tiles over DFF
EPS = 1e-3
BLOCKS = [[0, 3, 4, 7], [1, 2, 5, 6]]   # 128-row q-blocks per half-core

# element offsets into the flat bf16 weight pack
_WOFF = {}
_off = 0
for _nm, _sz in (("wq1", D * D), ("wk1", D * D), ("wv1", D * D), ("wo1", D * D),
                 ("wq2", D * D), ("wk2", D * D), ("wv2", D * D), ("wo2", D * D),
                 ("w_ff1", D * DFF), ("w_ff2", DFF * D)):
    _WOFF[_nm] = _off
    _off += _sz
WPACK_N = _off

# element offsets into the flat f32 param pack
_VOFF = {}
_off = 0
for _nm, _sz in (("bq1", D), ("bk1", D), ("bo1", D),
                 ("bq2", D), ("bk2", D), ("bo2", D),
                 ("bv1", D), ("bv2", D),
                 ("b_ff1", DFF), ("b_ff2", D),
                 ("g1", D), ("be1", D), ("g2", D), ("be2", D),
                 ("g3", D), ("be3", D)):
    _VOFF[_nm] = _off
    _off += _sz
VPACK_N = _off
# the pack ships transposed ([p, n] layout, contiguous per partition) plus a
# contiguous [bv1|bv2] appendix for the partition-broadcast rows
VPACK2_N = VPACK_N + 2 * D

_NC_CACHE = {}
DEBUG_TAPS = False


def _ln_stats(nc, pools, psln, m, y_m, ones_bf):
    """Accumulate sum / sq-sum of one [128, R] y tile into psln."""
    nc.tensor.matmul(psln[:, 0, :], ones_bf, y_m,
                     start=(m == 0), stop=(m == NK - 1))
    sq = pools["sq"].tile([128, R], BF16, tag="ln_sqt", name="ln_sqt")
    eng = nc.vector if m % 2 == 0 else nc.gpsimd
    eng.tensor_mul(sq, y_m, y_m)
    nc.tensor.matmul(psln[:, 1, :], ones_bf, sq,
                     start=(m == 0), stop=(m == NK - 1))


def _ln_finish(nc, pools, psln, y, gcol, bcol, out_tiles):
    """Finish LayerNorm from accumulated stats; write out_tiles per m."""
    tmp = pools["lntmp"]
    pssum, pssq = psln[:, 0, :], psln[:, 1, :]
    mean = tmp.tile([128, R], F32, tag="ln_mean", name="ln_mean")
    nc.vector.tensor_scalar_mul(mean, pssum, 1.0 / D)
    rv = tmp.tile([128, R], F32, tag="ln_sc", name="ln_rv")
    nc.vector.tensor_scalar_mul(rv, pssq, 1.0 / D)      # E[x^2]
    msq = tmp.tile([128, R], F32, tag="ln_t0", name="ln_msq")
    nc.vector.tensor_mul(msq, mean, mean)
    nc.vector.tensor_sub(rv, rv, msq)                   # var
    nc.scalar.activation(rv, rv, AF.Sqrt, bias=pools["epsc"][:, 0:1])
    nc.vector.reciprocal(rv, rv)                        # rstd (broadcast)
    nc.vector.tensor_mul(mean, mean, rv)                # mean*rstd (broadcast)
    # per-tile normalize split across vector (even m) and gpsimd (odd m);
    # the gamma/beta affine rides on the scalar engine so the two ALU
    # engines only do 2 ops per tile
    for m in range(NK):
        eng = nc.vector if m % 2 == 0 else nc.gpsimd
        t = tmp.tile([128, R], F32, tag=f"ln_t{m % 2}", name="ln_t")
        eng.tensor_mul(t, y[m], rv)
        eng.tensor_sub(t, t, mean)
        nc.scalar.activation(out_tiles[m], t, AF.Identity,
                             bias=bcol[:, m:m + 1], scale=gcol[:, m:m + 1])


def _issue_w0(nc, pools, wv_h, wk_h, wq_h, label):
    """DMA the first weight half of an attention; call early to prefetch."""
    w0 = {}
    for nm, h in (("wv", wv_h), ("wk", wk_h), ("wq", wq_h)):
        t = pools["w"].tile([128, 4, NK, 128], BF16, tag="wq",
                            name=f"{nm}0_{label}")
        nc.sync.dma_start(out=t, in_=h[0])
        w0[nm] = t
    return w0


def _attention(nc, pools, q_src, kv_src, resid, w0, wq_h, wk_h, wv_h, wo_h,
               bqc, bkc, bv_row, boc, mask_sb, y_out, ones_row, ones_bf,
               label, prefetch_cb=None):
    """One multi-head attention block + residual; writes pre-LN y_out tiles
    and accumulates LN stats into a psln tile (returned) as each y tile is
    produced.

    w0: pre-issued {wv,wk,wq} tiles for m-blocks 0..3 (from _issue_w0).
    wq_h/wk_h/wv_h/wo_h: per-projection "half" APs — lists of 2 DRAM views
    [128(p), 4(m), NK(k), 128(c)] covering m-blocks {4i..4i+3}.
    bv_row: [1, D] f32r SBUF row of the V bias.
    """
    wpool, ps, tmp = pools["w"], pools["psum"], pools["atmp"]
    causal = mask_sb is not None

    # persistent V_aug; 4 pair slots so V production can run one pair-block
    # ahead of consumption. layout [128, pair%4, s(NK), 256] with per-s
    # columns [v_h0(64) | ones(128) | v_h1(64)]
    vaug = pools["vaug"].tile([128, 4, NK, 256], BF16, tag="vaug",
                              name=f"vaug_{label}")
    nc.vector.memset(vaug[:, :, :, 64:192], 1.0)

    # V bias broadcast to all partitions: bvrep[p, c] = bv[c]
    bvrep = pools["bvr"].tile([128, D], BF16, tag="bvrep", name=f"bvr_{label}")
    for half in range(2):
        psb = ps.tile([128, R], F32, tag="ps_gen", name="ps_bv")
        nc.tensor.matmul(psb, ones_row, bv_row[:, half * 512:(half + 1) * 512],
                         start=True, stop=True)
        nc.vector.tensor_copy(bvrep[:, half * 512:(half + 1) * 512], psb)

    state = {"wv0": w0["wv"], "wk0": w0["wk"], "wq0": w0["wq"]}

    def emit_vk(jj):
        """V for pairs {2jj, 2jj+1} into vaug, K^T tiles for those pairs."""
        i, jl = jj // 2, jj % 2
        if jj == 1:
            # prefetch the second weight half one pair-block early
            for nm, hh in (("wv1", wv_h), ("wk1", wk_h), ("wq1", wq_h)):
                t = wpool.tile([128, 4, NK, 128], BF16, tag="wq",
                               name=f"{nm}_{label}")
                nc.sync.dma_start(out=t, in_=hh[1])
                state[nm] = t
        wv_cur, wk_cur = state[f"wv{i}"], state[f"wk{i}"]
        # ---- V for the two pairs produced together (moving dim 256) ----
        bvv = bvrep[:, jj * 256:(jj + 1) * 256].rearrange(
            "p (i a c) -> p i a c", a=2, c=64)
        for s in range(NK):
            psv = ps.tile([128, 256], F32, tag="ps_gen", name="ps_v")
            for k in range(NK):
                nc.tensor.matmul(psv, kv_src[k][:, s * 128:(s + 1) * 128],
                                 wv_cur[:, 2 * jl:2 * jl + 2, k, :],
                                 start=(k == 0), stop=(k == NK - 1))
            # single fused copy+bias drops both pairs' V around the ones
            sl = (2 * jj) % 4
            dst = vaug[:, sl:sl + 2, s, :].rearrange(
                "p i (a c) -> p i a c", c=64)[:, :, 0:4:3, :]
            src = psv.rearrange("p (i a c) -> p i a c", a=2, c=64)
            nc.vector.tensor_add(dst, src, bvv)
        # ---- K^T for the two pairs ----
        for p in (2 * jj, 2 * jj + 1):
            pl = p % 4
            kt_p = pools["kt"].tile([128, S], BF16, tag="kt",
                                    name=f"kt_{label}_{p}")
            for half in range(2):
                psk = ps.tile([128, R], F32, tag="ps_gen", name="ps_k")
                for k in range(NK):
                    nc.tensor.matmul(psk, wk_cur[:, pl, k, :],
                                     kv_src[k][:, half * 512:(half + 1) * 512],
                                     start=(k == 0), stop=(k == NK - 1))
                nc.scalar.activation(kt_p[:, half * 512:(half + 1) * 512], psk,
                                     AF.Identity, bias=bkc[:, p:p + 1])
            state[f"kt{p}"] = kt_p

    emit_vk(0)
    nh = []
    for p in range(NP):
        if p % 2 == 0 and p // 2 + 1 <= 3:
            emit_vk(p // 2 + 1)        # keep V/K one pair-block ahead
        pl = p % 4
        kt_p = state.pop(f"kt{p}")

        # ---- Q^T for this pair ----
        psq = ps.tile([128, R], F32, tag="ps_gen", name="ps_q")
        for k in range(NK):
            nc.tensor.matmul(psq, state[f"wq{p // 4}"][:, pl, k, :], q_src[k],
                             start=(k == 0), stop=(k == NK - 1))
        qt_p = pools["qt"].tile([128, R], BF16, tag="qt", name=f"qt_{label}_{p}")
        nc.scalar.activation(qt_p, psq, AF.Identity, bias=bqc[:, p:p + 1])

        # ---- scores^T, exp, attnV (softmax sums ride along in V_aug ones) --
        # Both heads share one 2-bank PSUM tile; exp runs once over both.
        va = vaug[:, p % 4]
        psap = pools["psatt"].tile([128, 2, R], F32, tag="ps_att", name="ps_att")
        for s in range(NK):
            c0 = 128 * (s // 2) if causal else 0
            sscp = pools["pssc"].tile([128, 2, R], F32, tag="ps_sc", name="ps_sc")
            nc.tensor.matmul(sscp[:, 0, c0:R], kt_p[0:64, s * 128:(s + 1) * 128],
                             qt_p[0:64, c0:R], start=True, stop=True)
            nc.tensor.matmul(sscp[:, 1, c0:R], kt_p[64:128, s * 128:(s + 1) * 128],
                             qt_p[64:128, c0:R], start=True, stop=True)
            e = pools["exp"].tile([128, 2, R], BF16, tag="e", name="e")
            nc.scalar.activation(e[:, :, c0:R], sscp[:, :, c0:R], AF.Exp,
                                 scale=0.125)
            if causal:
                # zero the future half of the diagonal q-block (binary mask,
                # duplicated per head) on the otherwise-idle GpSimd engine
                nc.gpsimd.tensor_mul(
                    e[:, :, c0:c0 + 128], e[:, :, c0:c0 + 128],
                    mask_sb[:, s, :].rearrange("p (h c) -> p h c", h=2))
            nc.tensor.matmul(psap[:, 0, c0:R], va[:, s, 0:128], e[:, 0, c0:R],
                             start=(s == 0), stop=(s == NK - 1),
                             skip_group_check=True)
            nc.tensor.matmul(psap[:, 1, c0:R], va[:, s, 128:256], e[:, 1, c0:R],
                             start=(s == 0), stop=(s == NK - 1),
                             skip_group_check=True)

        # evacuate the attnV accumulator promptly so the next pair's attnV can
        # reuse the psatt banks without waiting for the normalize chain
        pcp = pools["sq"].tile([128, 2, R], F32, tag="pcp", name="pcp")
        nc.vector.tensor_copy(pcp, psap)

        # normalize: head0 out rows 0:64 / sums 64:128; head1 sums 0:64 / out 64:128.
        # Bounce both heads' sums into out-aligned partitions, then a single
        # reciprocal covers both heads (DVE reciprocal cost is per-column).
        nh_p = pools["nh"].tile([128, R], BF16, tag=f"nh_{p}", name=f"nh_{label}_{p}")
        sums = tmp.tile([128, R], F32, tag="inv", name="sums")
        nc.sync.dma_start(out=sums[0:64, :], in_=pcp[64:128, 0, :])
        nc.sync.dma_start(out=sums[64:128, :], in_=pcp[0:64, 1, :])
        inv = tmp.tile([128, R], F32, tag="invs", name="inv")
        nc.vector.reciprocal(inv, sums)
        # gpsimd is mask-bound in the causal block; use it only when free
        meng = nc.vector if causal else nc.gpsimd
        meng.tensor_mul(nh_p[0:64, :], pcp[0:64, 0, :], inv[0:64, :])
        meng.tensor_mul(nh_p[64:128, :], pcp[64:128, 1, :], inv[64:128, :])
        nh.append(nh_p)
        if p == 4:
            state["wo0"] = wpool.tile([128, 4, NK, 128], BF16, tag="wo",
                                      bufs=2, name=f"woc_{label}_0")
            nc.sync.dma_start(out=state["wo0"], in_=wo_h[0])
        if p == 5:
            state["wo1"] = wpool.tile([128, 4, NK, 128], BF16, tag="wo",
                                      bufs=2, name=f"woc_{label}_1")
            nc.sync.dma_start(out=state["wo1"], in_=wo_h[1])

    if prefetch_cb is not None:
        prefetch_cb()   # issue the next phase's first weight DMAs now

    # ---- output projection + bias + residual (+ LN stats per tile) ----
    psln = pools["psatt"].tile([128, 2, R], F32, tag="ps_att",
                               name=f"ln_ps_{label}")
    for m in range(NK):
        wo_cur = state["wo0"] if m < 4 else state["wo1"]
        pso = ps.tile([128, R], F32, tag="ps_gen", name="ps_o")
        for p in range(NP):
            nc.tensor.matmul(pso, wo_cur[:, m % 4, p, :], nh[p],
                             start=(p == 0), stop=(p == NP - 1))
        nc.vector.scalar_tensor_tensor(y_out[m], pso, boc[:, m:m + 1],
                                       resid[m], ALU.add, ALU.add)
        _ln_stats(nc, pools, psln, m, y_out[m], ones_bf)
    return psln


def build_nc(reps=1):
    nc = bacc.Bacc("TRN2", target_bir_lowering=False, debug=False)

    xt = nc.dram_tensor("xt", [D, S], BF16, kind="ExternalInput")
    xq = nc.dram_tensor("xq", [D, R], BF16, kind="ExternalInput")
    enc = nc.dram_tensor("enc", [D, S], BF16, kind="ExternalInput")
    maskst = nc.dram_tensor("maskst", [128, NK, 256], BF16, kind="ExternalInput")
    wpack = nc.dram_tensor("wpack", [WPACK_N], BF16, kind="ExternalInput")
    vpack = nc.dram_tensor("vpack", [VPACK2_N], F32, kind="ExternalInput")
    out_t = nc.dram_tensor("out_t", [D, R], BF16, kind="ExternalOutput")

    def whalves(nm, nm_blocks):
        # host layout [m, p, k, c]; -> list of [128(p), 4(m), NK(k), 128(c)]
        full = wpack[_WOFF[nm]:_WOFF[nm] + nm_blocks * D * 128].rearrange(
            "(m p k c) -> p m k c", p=128, k=NK, c=128)
        return [full[:, 4 * i:4 * i + 4] for i in range(nm_blocks // 4)]

    def ff2_views():
        # host layout [m, q, p, k, c]; -> per m: [128, 4(q), NK, 128]
        full = wpack[_WOFF["w_ff2"]:_WOFF["w_ff2"] + DFF * D].rearrange(
            "(m q p k c) -> p m q k c", q=DFF // D, p=128, k=NK, c=128)
        return [full[:, m] for m in range(NK)]

    from contextlib import ExitStack
    with tile.TileContext(nc) as tc, ExitStack() as ctx:
        pools = {
            "const": ctx.enter_context(tc.tile_pool(name="const", bufs=1)),
            "w": ctx.enter_context(tc.tile_pool(name="wpool", bufs=4)),
            "psum": ctx.enter_context(tc.tile_pool(name="pspool", bufs=2, space="PSUM")),
            "psatt": ctx.enter_context(tc.tile_pool(name="psatt", bufs=1, space="PSUM")),
            "pssc": ctx.enter_context(tc.tile_pool(name="pssc", bufs=2, space="PSUM")),
            "lntmp": ctx.enter_context(tc.tile_pool(name="lntmp", bufs=1)),
            "sq": ctx.enter_context(tc.tile_pool(name="sqpool", bufs=2)),
            "acts": ctx.enter_context(tc.tile_pool(name="acts", bufs=1)),
            "qt": ctx.enter_context(tc.tile_pool(name="qt", bufs=2)),
            "kt": ctx.enter_context(tc.tile_pool(name="kt", bufs=4)),
            "vaug": ctx.enter_context(tc.tile_pool(name="vaug", bufs=1)),
            "bvr": ctx.enter_context(tc.tile_pool(name="bvr", bufs=1)),
            "nh": ctx.enter_context(tc.tile_pool(name="nh", bufs=1)),
            "exp": ctx.enter_context(tc.tile_pool(name="exp", bufs=2)),
            "atmp": ctx.enter_context(tc.tile_pool(name="atmp", bufs=2)),
            "amask": ctx.enter_context(tc.tile_pool(name="amask", bufs=1)),
            "hpool": ctx.enter_context(tc.tile_pool(name="hpool", bufs=1)),
        }
        const = pools["const"]
        acts = pools["acts"]

        wff1_h = whalves("w_ff1", NM2)
        wff2_v = ff2_views()

        for _rep in range(reps):
            # ---- big DMAs first: dma_start issue is ~1us each on SP ----
            # xt split by key halves so a1's first V matmuls start sooner
            xt_all = acts.tile([128, NK, S], BF16, tag="kv", name="xt_all")
            xt_v = xt.rearrange("(k p) s -> p k s", p=128)
            nc.sync.dma_start(out=xt_all[:, :, 0:512], in_=xt_v[:, :, 0:512])
            nc.sync.dma_start(out=xt_all[:, :, 512:S], in_=xt_v[:, :, 512:S])
            xq_all = acts.tile([128, NK, R], BF16, tag="xq", name="xq_all")
            nc.sync.dma_start(out=xq_all, in_=xq.rearrange("(k p) s -> p k s", p=128))
            wq1_h, wk1_h, wv1_h = whalves("wq1", NK), whalves("wk1", NK), whalves("wv1", NK)
            w0_a1 = _issue_w0(nc, pools, wv1_h, wk1_h, wq1_h, "a1")

            if _rep == 0:
                # ---- all small fp32 params in one DMA ----
                vall = const.tile([128, VPACK_N // 128], F32, tag="vall", name="vall")
                nc.sync.dma_start(out=vall,
                                  in_=vpack[0:VPACK_N].rearrange("(p n) -> p n", p=128))
                bias_cols = {nm: vall[:, _VOFF[nm] // 128:_VOFF[nm] // 128 + NK]
                             for nm in ("bq1", "bk1", "bo1", "bq2", "bk2", "bo2",
                                        "b_ff2", "g1", "be1", "g2", "be2",
                                        "g3", "be3")}
                bff1c = vall[:, _VOFF["b_ff1"] // 128:_VOFF["b_ff1"] // 128 + NM2]
                # V biases as rows (for partition-broadcast via K=1 matmul);
                # the f32 -> f32r rounding happens inside the DMA
                bvrow = const.tile([1, 2 * D], F32R, tag="bvrow", name="bvrow")
                nc.sync.dma_start(
                    out=bvrow,
                    in_=vpack[VPACK_N:VPACK2_N].rearrange(
                        "(a c) -> a c", a=1).bitcast(F32R))
                bv_rows = {1: bvrow[0:1, 0:D], 2: bvrow[0:1, D: 2 * D]}
                mask_sb = pools["amask"].tile([128, NK, 256], BF16, tag="mask",
                                              name="mask")
                nc.sync.dma_start(out=mask_sb, in_=maskst[:, :, :])
                ones_bf = const.tile([128, 128], BF16, tag="ones_bf", name="ones_bf")
                nc.vector.memset(ones_bf, 1.0)
                ones_f = const.tile([1, 128], F32, tag="ones_f", name="ones_f")
                nc.vector.memset(ones_f, 1.0)
                ones_row = ones_f.bitcast(F32R)[0:1, :]
                epsc = const.tile([128, 1], F32, tag="epsc", name="epsc")
                nc.vector.memset(epsc, EPS)
                pools["epsc"] = epsc

            enc_all = acts.tile([128, NK, S], BF16, tag="env", name="enc_all")
            nc.sync.dma_start(out=enc_all, in_=enc.rearrange("(k p) s -> p k s", p=128))

            xt_sb = [xt_all[:, k, :] for k in range(NK)]
            xq_sb = [xq_all[:, k, :] for k in range(NK)]
            enc_sb = [enc_all[:, k, :] for k in range(NK)]

            # o2 reuses the kv slot (xt is dead once a1's K/V are computed)
            o2_all = acts.tile([128, NK, R], BF16, tag="kv", name="o2_all")
            o2 = [o2_all[:, m, :] for m in range(NK)]

            # ================= attention =================
            wq2_h, wk2_h, wv2_h = (whalves("wq2", NK), whalves("wk2", NK),
                                   whalves("wv2", NK))
            w0_a2 = {}

            def prefetch_a2():
                w0_a2.update(_issue_w0(nc, pools, wv2_h, wk2_h, wq2_h, "a2"))

            y1 = [acts.tile([128, R], BF16, tag=f"y_{m}", name=f"y1_{m}")
                  for m in range(NK)]
            psln1 = _attention(nc, pools, xq_sb, xt_sb, xq_sb, w0_a1,
                               wq1_h, wk1_h, wv1_h, whalves("wo1", NK),
                               bias_cols["bq1"], bias_cols["bk1"],
                               bv_rows[1], bias_cols["bo1"], mask_sb, y1,
                               ones_row, ones_bf, "a1", prefetch_cb=prefetch_a2)
            # o1 reuses the xq slot (xq is dead once y1 is written)
            o1_all = acts.tile([128, NK, R], BF16, tag="xq", name="o1_all")
            o1 = [o1_all[:, m, :] for m in range(NK)]
            _ln_finish(nc, pools, psln1, y1, bias_cols["g1"], bias_cols["be1"], o1)
            if DEBUG_TAPS:
                dbg_y1 = nc.dram_tensor("dbg_y1", [128, NK, R], BF16,
                                        kind="ExternalOutput")
                for m in range(NK):
                    nc.sync.dma_start(out=dbg_y1[:, m, :], in_=y1[m])
                dbg_o1 = nc.dram_tensor("dbg_o1", [128, NK, R], BF16,
                                        kind="ExternalOutput")
                nc.sync.dma_start(out=dbg_o1[:, :, :], in_=o1_all)

            wt_next = None

            def prefetch_ffn():
                nonlocal wt_next
                wt_next = pools["w"].tile([128, 4, NK, 128], BF16, tag="wq",
                                          name="wff1_0")
                nc.sync.dma_start(out=wt_next, in_=wff1_h[0])

            y2 = [acts.tile([128, R], BF16, tag=f"y_{m}", name=f"y2_{m}")
                  for m in range(NK)]
            psln2 = _attention(nc, pools, o1, enc_sb, o1, w0_a2,
                               wq2_h, wk2_h, wv2_h, whalves("wo2", NK),
                               bias_cols["bq2"], bias_cols["bk2"],
                               bv_rows[2], bias_cols["bo2"], None, y2,
                               ones_row, ones_bf, "a2", prefetch_cb=prefetch_ffn)
            _ln_finish(nc, pools, psln2, y2, bias_cols["g2"], bias_cols["be2"], o2)
            if DEBUG_TAPS:
                dbg_y2 = nc.dram_tensor("dbg_y2", [128, NK, R], BF16,
                                        kind="ExternalOutput")
                for m in range(NK):
                    nc.sync.dma_start(out=dbg_y2[:, m, :], in_=y2[m])
                dbg_o2 = nc.dram_tensor("dbg_o2", [128, NK, R], BF16,
                                        kind="ExternalOutput")
                for m in range(NK):
                    nc.sync.dma_start(out=dbg_o2[:, m, :], in_=o2[m])

            # ================= FFN =================
            h = []
            wt = None
            for m in range(NM2):
                if m % 4 == 0:
                    wt = wt_next
                    if m // 4 + 1 < NM2 // 4:
                        wt_next = pools["w"].tile([128, 4, NK, 128], BF16,
                                                  tag="wq", name=f"wff1_{m // 4 + 1}")
                        nc.sync.dma_start(out=wt_next, in_=wff1_h[m // 4 + 1])
                    elif m // 4 + 1 == NM2 // 4:
                        wt_next = pools["w"].tile([128, 4, NK, 128], BF16,
                                                  tag="wq", name="wff2_0")
                        nc.sync.dma_start(out=wt_next, in_=wff2_v[0])
                psh = pools["psum"].tile([128, R], F32, tag="ps_gen", name="ps_h")
                for k in range(NK):
                    nc.tensor.matmul(psh, wt[:, m % 4, k, :], o2[k],
                                     start=(k == 0), stop=(k == NK - 1))
                h_m = pools["hpool"].tile([128, R], BF16, tag=f"h_{m}", name=f"h_{m}")
                nc.scalar.activation(h_m, psh, AF.Relu, bias=bff1c[:, m:m + 1])
                h.append(h_m)

            y3 = [acts.tile([128, R], BF16, tag=f"y_{m}", name=f"y3_{m}")
                  for m in range(NK)]
            psln3 = pools["psatt"].tile([128, 2, R], F32, tag="ps_att", name="ln3_ps")
            for m in range(NK):
                psf = pools["psum"].tile([128, R], F32, tag="ps_gen", name="ps_f")
                wt = wt_next
                if m + 1 < NK:
                    wt_next = pools["w"].tile([128, 4, NK, 128], BF16, tag="wq",
                                              name=f"wff2_{m + 1}")
                    nc.sync.dma_start(out=wt_next, in_=wff2_v[m + 1])
                for q in range(4):
                    for k in range(NK):
                        nc.tensor.matmul(psf, wt[:, q, k, :], h[q * NK + k],
                                         start=(q == 0 and k == 0),
                                         stop=(q == 3 and k == NK - 1))
                nc.vector.scalar_tensor_tensor(y3[m], psf, bias_cols["b_ff2"][:, m:m + 1],
                                               o2[m], ALU.add, ALU.add)
                _ln_stats(nc, pools, psln3, m, y3[m], ones_bf)
            # o3 reuses the xq slot (o1 is dead once y2 is written)
            o3_all = acts.tile([128, NK, R], BF16, tag="xq", name="o3_all")
            o3 = [o3_all[:, m, :] for m in range(NK)]
            _ln_finish(nc, pools, psln3, y3, bias_cols["g3"], bias_cols["be3"], o3)
            out_v = out_t.rearrange("(m p) r -> p m r", p=128)
            for m in range(NK):
                nc.sync.dma_start(out=out_v[:, m, :], in_=o3_all[:, m, :])

    nc.compile()
    return nc


def _get_nc(reps=1):
    if reps not in _NC_CACHE:
        _NC_CACHE[reps] = build_nc(reps)
    return _NC_CACHE[reps]


def _pack_weights(inputs):
    bf = ml_dtypes.bfloat16
    wpack = np.empty((WPACK_N,), dtype=bf)

    def put(nm, arr):
        a = arr.reshape(-1)
        wpack[_WOFF[nm]:_WOFF[nm] + a.size] = a

    for nm in ("wq1", "wk1", "wv1", "wo1", "wq2", "wk2", "wv2", "wo2"):
        w = np.asarray(inputs[nm], dtype=np.float32).astype(bf)
        # [k*128+p, m*128+c] -> [m, p, k, c]
        put(nm, w.reshape(NK, 128, NK, 128).transpose(2, 1, 0, 3))
    w = np.asarray(inputs["w_ff1"], dtype=np.float32).astype(bf)
    put("w_ff1", w.reshape(NK, 128, NM2, 128).transpose(2, 1, 0, 3))
    w = np.asarray(inputs["w_ff2"], dtype=np.float32).astype(bf)
    # [q*8*128 + k*128 + p, m*128+c] -> [m, q, p, k, c]
    put("w_ff2", w.reshape(DFF // D, NK, 128, NK, 128).transpose(3, 0, 2, 1, 4))

    vlin = np.empty((VPACK_N,), dtype=np.float32)
    for nm in ("bq1", "bk1", "bo1", "bq2", "bk2", "bo2", "bv1", "bv2",
               "b_ff1", "b_ff2", "g1", "be1", "g2", "be2", "g3", "be3"):
        a = np.asarray(inputs[nm], dtype=np.float32).reshape(-1)
        vlin[_VOFF[nm]:_VOFF[nm] + a.size] = a
    # ship transposed ([p, n], contiguous per partition) + contiguous bv rows
    vpack = np.empty((VPACK2_N,), dtype=np.float32)
    vpack[0:VPACK_N] = np.ascontiguousarray(
        vlin.reshape(VPACK_N // 128, 128).T).reshape(-1)
    vpack[VPACK_N:] = vlin[_VOFF["bv1"]:_VOFF["bv1"] + 2 * D]
    return wpack, vpack


def _make_in_maps(inputs):
    full_k = np.arange(S)
    wpack, vpack = _pack_weights(inputs)
    in_maps = []
    metas = []
    for c in range(8):
        b, half = c // 2, c % 2
        qidx = np.concatenate([np.arange(128) + 128 * blk for blk in BLOCKS[half]])
        xt_b = np.ascontiguousarray(np.asarray(inputs["inputs"][b]).T.astype(ml_dtypes.bfloat16))
        enc_b = np.ascontiguousarray(np.asarray(inputs["enc_outputs"][b]).T.astype(ml_dtypes.bfloat16))
        xq_b = np.ascontiguousarray(xt_b[:, qidx])
        mask = np.where(full_k[:, None] <= qidx[None, :], 1.0, 0.0).astype(ml_dtypes.bfloat16)
        mask8 = np.stack([mask[s * 128:(s + 1) * 128, 128 * (s // 2):128 * (s // 2) + 128]
                          for s in range(NK)])           # [s, p, c]
        mask8 = np.concatenate([mask8, mask8], axis=-1)  # duplicate per head
        mask8 = np.ascontiguousarray(mask8.transpose(1, 0, 2))   # [p, s, 2*c]
        m = {"xt": xt_b, "xq": xq_b, "enc": enc_b,
             "maskst": mask8,
             "wpack": wpack, "vpack": vpack}
        in_maps.append(m)
        metas.append((b, qidx))
    return in_maps, metas


def kernel(**inputs):
    nc = _get_nc()
    in_maps, metas = _make_in_maps(inputs)
    res = run_bass_kernel_spmd(nc, in_maps, core_ids=list(range(8)))
    out = np.zeros((B, S, D), dtype=np.float32)
    for c, (b, qidx) in enumerate(metas):
        out[b, qidx, :] = res.results[c]["out_t"].astype(np.float32).T
    return out
